# revision 1
# baseline (speedup 1.0000x reference)
"""DGCNN (3x EdgeConv + pointwise projection) Trainium2 Bass kernel.

Problem: x [8, 4096, 3] -> out [8, 4096, 64]; k=20 kNN per EdgeConv.

Math reformulation (per sample, per EdgeConv with weight w = [W1; W2], bias b):
  h_ij = LeakyReLU(x_i @ W1 + (x_j - x_i) @ W2 + b)      j in kNN(i)
       = LeakyReLU(p_i + q_j),  p = x@(W1-W2)+b, q = x@W2
  out_i = max_j h_ij = LeakyReLU(p_i + max_j q_j)        (LeakyReLU monotone)

Per EdgeConv: (1) distance matmul on PE (augmented: lhsT=[2x^T;1],
rhs=[x^T;-|x|^2], ranking-equivalent to the reference's neg_dist),
(2) exact top-20 per row on DVE via 3x (max8 + max_index + match_replace),
(3) 20 single-offset-per-partition indirect DMA gathers of q rows + max fold,
(4) p + m -> LeakyReLU (exact, via max(x, 0.01x) on DVE).

Sharding: data-parallel over batch, 1 sample per NeuronCore (8 cores).
This environment is a bedrock image: Q7 GPSIMD library instructions
(dma_gather/ap_gather/iota/pool tensor ops) are unavailable; only
[P,1]-offset indirect DMA works for gathers. Constants (identity) ship
from the host.
"""

import sys

sys.path.insert(0, "/opt/trn_rl_repo")

import numpy as np

import concourse.bass as bass
import concourse.bacc as bacc
import concourse.mybir as mybir
import concourse.tile as tile

N = 4096          # points per sample
P = 128           # partition tile (points per row-tile)
RT = N // P       # 32 row-tiles
KNN = 20          # neighbors
NB = 24           # extracted per row (3 batches of 8)
CO = 64           # feature channels
F32 = mybir.dt.float32
U16 = mybir.dt.uint16
I32 = mybir.dt.int32

AL = mybir.AluOpType
AX = mybir.AxisListType
NEG = -1e30


def _edge_conv(nc, tc, cst, pools, A, Bt, rq, rp, q_dram, xTn, lay, proj=None):
    """One EdgeConv layer. A [Kd, N] = [2x^T; 1], Bt [Kd, N] = [x^T; -sq].
    rq/rp [Kd, CO] rhs for q/p matmuls. Writes xTn [CO, N] = LeakyReLU(p+m)^T.
    If proj is given (final layer), also emits the projection per row-tile."""
    sb, sm, ps_s, ps_sm = pools

    # ---- head: q = x @ W2 row-major into q_dram [N, CO]
    for i in range(RT):
        isl = slice(i * P, (i + 1) * P)
        qp = ps_sm.tile([P, CO], F32, tag="sm")
        nc.tensor.matmul(qp[:], lhsT=A[:, isl], rhs=rq[:], start=True, stop=True)
        qs = sm.tile([P, CO], F32, tag="qs")
        nc.scalar.copy(qs[:], qp[:])
        nc.sync.dma_start(q_dram[isl, :], qs[:])

    # ---- row-tile loop
    for i in range(RT):
        isl = slice(i * P, (i + 1) * P)

        # distances s_I [P, N] via PE in 4 PSUM quarters -> SBUF
        s_sb = sb.tile([P, N], F32, tag="s_sb")
        for qi in range(4):
            pq = ps_s.tile([P, 1024], F32, tag="ps_s")
            for c in range(2):
                col = qi * 1024 + c * 512
                nc.tensor.matmul(
                    pq[:, c * 512:(c + 1) * 512],
                    lhsT=A[:, isl],
                    rhs=Bt[:, col:col + 512],
                    start=True, stop=True,
                )
            nc.scalar.copy(s_sb[:, qi * 1024:(qi + 1) * 1024], pq[:])

        # exact top-24 (values desc) with global indices
        gidx = sm.tile([P, NB], U16, tag="gidx")
        cv8 = sm.tile([P, 8], F32, tag="cv8")
        for b in range(3):
            nc.vector.max(out=cv8[:], in_=s_sb[:])
            nc.vector.max_index(
                out=gidx[:, b * 8:(b + 1) * 8], in_max=cv8[:], in_values=s_sb[:]
            )
            nc.vector.match_replace(
                out=s_sb[:], in_to_replace=cv8[:], in_values=s_sb[:], imm_value=NEG
            )
        j32 = sm.tile([P, KNN], I32, tag="j32")
        nc.vector.tensor_copy(j32[:], gidx[:, 0:KNN])

        # gather q rows of the top-20 neighbors (one offset per partition each)
        gath = sm.tile([P, KNN, CO], F32, tag="gath")
        for k in range(KNN):
            nc.gpsimd.indirect_dma_start(
                out=gath[:, k, :], out_offset=None, in_=q_dram[:],
                in_offset=bass.IndirectOffsetOnAxis(ap=j32[:, k:k + 1], axis=0),
            )
        # max fold over k
        for (a0, a1, n) in ((0, 10, 10), (0, 5, 5), (0, 2, 2), (0, 1, 1)):
            nc.vector.tensor_tensor(
                out=gath[:, a0:a0 + n, :], in0=gath[:, a0:a0 + n, :],
                in1=gath[:, a1:a1 + n, :], op=AL.max,
            )
        nc.vector.tensor_tensor(
            out=gath[:, 0:1, :], in0=gath[:, 0:1, :], in1=gath[:, 4:5, :], op=AL.max,
        )

        # h = LeakyReLU(p + m)
        pp = ps_sm.tile([P, CO], F32, tag="sm")
        nc.tensor.matmul(pp[:], lhsT=A[:, isl], rhs=rp[:], start=True, stop=True)
        hs = sm.tile([P, CO], F32, tag="hs")
        nc.vector.tensor_tensor(
            out=hs[:], in0=pp[:],
            in1=gath[:].rearrange("p a b -> p (a b)")[:, 0:CO], op=AL.add,
        )
        h = sm.tile([P, CO], F32, tag="h")
        nc.vector.scalar_tensor_tensor(
            out=h[:], in0=hs[:], scalar=0.01, in1=hs[:], op0=AL.mult, op1=AL.max,
        )

        # transpose into xTn[:, isl]
        tp = ps_sm.tile([CO, P], F32, tag="sm")
        nc.tensor.transpose(out=tp[:], in_=h[:], identity=cst["ident"][:])
        nc.scalar.copy(xTn[:, isl], tp[:])

        if proj is not None:
            out_dram, A1, x1T, x2T, wpx, wp1, wp2, wp3 = proj
            pj = ps_sm.tile([P, CO], F32, tag="sm")
            nc.tensor.matmul(pj[:], lhsT=A1[:, isl], rhs=wpx[:], start=True, stop=False)
            nc.tensor.matmul(pj[:], lhsT=x1T[:, isl], rhs=wp1[:], start=False, stop=False)
            nc.tensor.matmul(pj[:], lhsT=x2T[:, isl], rhs=wp2[:], start=False, stop=False)
            nc.tensor.matmul(pj[:], lhsT=xTn[:, isl], rhs=wp3[:], start=False, stop=True)
            po = sm.tile([P, CO], F32, tag="po")
            nc.scalar.copy(po[:], pj[:])
            nc.sync.dma_start(out_dram[isl, :], po[:])


def _build_a_bt(nc, tc, cst, pools, xT, A, Bt):
    """A = [2 xT; ones], Bt = [xT; -sum(xT^2, axis=C)] for a [CO, N] xT."""
    sb, sm, ps_s, ps_sm = pools
    nc.scalar.mul(A[0:CO, :], xT[:], 2.0)
    nc.vector.memset(A[CO:CO + 1, :], 1.0)
    nc.scalar.copy(Bt[0:CO, :], xT[:])
    xsq = sb.tile([CO, N], F32, tag="xsq")
    nc.scalar.square(xsq[:], xT[:])
    for c in range(8):
        sq = ps_sm.tile([1, 512], F32, tag="sm")
        nc.tensor.matmul(
            sq[:], lhsT=cst["ones64"][:], rhs=xsq[:, c * 512:(c + 1) * 512],
            start=True, stop=True,
        )
        nc.scalar.mul(Bt[CO:CO + 1, c * 512:(c + 1) * 512], sq[:], -1.0)


def build_nc():
    nc = bacc.Bacc("TRN2", target_bir_lowering=False, debug=False)

    ins = {}
    for name, shape in [
        ("A1", [4, N]), ("Bt1", [4, N]),
        ("rq1", [4, CO]), ("rp1", [4, CO]),
        ("rq2", [CO + 1, CO]), ("rp2", [CO + 1, CO]),
        ("rq3", [CO + 1, CO]), ("rp3", [CO + 1, CO]),
        ("wpx", [4, CO]), ("wp1", [CO, CO]), ("wp2", [CO, CO]), ("wp3", [CO, CO]),
        ("ident", [P, P]),
    ]:
        ins[name] = nc.dram_tensor(name, shape, F32, kind="ExternalInput")
    out_dram = nc.dram_tensor("out", [N, CO], F32, kind="ExternalOutput")

    q_drams = [
        nc.dram_tensor(f"q_scr{i}", [N, CO], F32, kind="Internal") for i in range(3)
    ]

    from contextlib import ExitStack

    with tile.TileContext(nc) as tc, ExitStack() as ctx:
        cpool = ctx.enter_context(tc.tile_pool(name="const", bufs=1))
        sb = ctx.enter_context(tc.tile_pool(name="sb", bufs=2))
        sm = ctx.enter_context(tc.tile_pool(name="sm", bufs=2))
        ps_s = ctx.enter_context(tc.tile_pool(name="ps_s", bufs=2, space="PSUM"))
        ps_sm = ctx.enter_context(tc.tile_pool(name="ps_sm", bufs=4, space="PSUM"))
        pools = (sb, sm, ps_s, ps_sm)

        cst = {}
        cst["ones64"] = cpool.tile([CO, 1], F32, tag="ones64", name="ones64")
        nc.vector.memset(cst["ones64"][:], 1.0)

        # load inputs to SBUF
        st = {}
        for name in ins:
            t = cpool.tile(list(ins[name].shape), F32, tag=f"in_{name}",
                           name=f"in_{name}")
            nc.sync.dma_start(t[:], ins[name][:])
            st[name] = t
        cst["ident"] = st["ident"]

        x1T = cpool.tile([CO, N], F32, tag="x1T", name="x1T")
        x2T = cpool.tile([CO, N], F32, tag="x2T", name="x2T")
        x3T = cpool.tile([CO, N], F32, tag="x3T", name="x3T")
        A = cpool.tile([CO + 1, N], F32, tag="A", name="A")
        Bt = cpool.tile([CO + 1, N], F32, tag="Bt", name="Bt")

        _edge_conv(nc, tc, cst, pools, st["A1"][:], st["Bt1"][:],
                   st["rq1"][:], st["rp1"][:], q_drams[0], x1T[:], 1)

        _build_a_bt(nc, tc, cst, pools, x1T[:], A[:], Bt[:])
        _edge_conv(nc, tc, cst, pools, A[:], Bt[:],
                   st["rq2"][:], st["rp2"][:], q_drams[1], x2T[:], 2)

        _build_a_bt(nc, tc, cst, pools, x2T[:], A[:], Bt[:])
        _edge_conv(nc, tc, cst, pools, A[:], Bt[:],
                   st["rq3"][:], st["rp3"][:], q_drams[2], x3T[:], 3,
                   proj=(out_dram, st["A1"][:], x1T[:], x2T[:],
                         st["wpx"][:], st["wp1"][:], st["wp2"][:], st["wp3"][:]))

    nc.compile()
    return nc


def host_prep(x, w1, b1, w2, b2, w3, b3, wp, bp):
    """Per-sample A1/Bt1 plus shared weight-derived rhs tensors."""
    f = np.float32
    B = x.shape[0]
    shared = {}
    for lay, (w, b) in enumerate([(w1, b1), (w2, b2), (w3, b3)], start=1):
        C = w.shape[0] // 2
        W1, W2 = w[:C], w[C:]
        rq = np.concatenate([W2 * 0.5, np.zeros((1, CO), f)], axis=0)
        rp = np.concatenate([(W1 - W2) * 0.5, b[None, :]], axis=0)
        shared[f"rq{lay}"] = np.ascontiguousarray(rq, f)
        shared[f"rp{lay}"] = np.ascontiguousarray(rp, f)
    shared["wpx"] = np.concatenate(
        [wp[0:3] * 0.5, np.zeros((1, CO), f)], axis=0
    ).astype(f)
    shared["wp1"] = np.ascontiguousarray(wp[3:67], f)
    shared["wp2"] = np.ascontiguousarray(wp[67:131], f)
    shared["wp3"] = np.ascontiguousarray(wp[131:195], f)
    shared["ident"] = np.eye(P, dtype=f)

    per_core = []
    for b_i in range(B):
        xT = np.ascontiguousarray(x[b_i].T, f)            # [3, N]
        sq = np.sum(xT * xT, axis=0, keepdims=True)       # [1, N]
        A1 = np.concatenate([2.0 * xT, np.ones((1, N), f)], axis=0).astype(f)
        Bt1 = np.concatenate([xT, -sq], axis=0).astype(f)
        m = dict(shared)
        m["A1"] = A1
        m["Bt1"] = Bt1
        per_core.append(m)
    return per_core


_NC_CACHE = {}
TRACE = [False]
LAST_RESULT = {}


def kernel(**inputs):
    from concourse.bass_utils import run_bass_kernel_spmd

    x = np.asarray(inputs["x"], np.float32)
    args = [np.asarray(inputs[n], np.float32) for n in
            ("w1", "b1", "w2", "b2", "w3", "b3", "wp", "bp")]
    bp = args[-1]
    assert int(inputs["k"]) == KNN

    in_maps = host_prep(x, *args)

    if "nc" not in _NC_CACHE:
        _NC_CACHE["nc"] = build_nc()
    nc = _NC_CACHE["nc"]

    res = run_bass_kernel_spmd(
        nc, in_maps, core_ids=list(range(len(in_maps))), trace=TRACE[0]
    )
    LAST_RESULT["res"] = res
    out = np.stack([r["out"] for r in res.results], axis=0)  # [B, N, CO]
    return (out + bp[None, None, :]).astype(np.float32)



# revision 7
# speedup vs baseline: 1.7919x; 1.7919x over previous
"""DGCNN (3x EdgeConv + pointwise projection) Trainium2 Bass kernel.

Problem: x [8, 4096, 3] -> out [8, 4096, 64]; k=20 kNN per EdgeConv.

Math reformulation (per sample, per EdgeConv with weight w = [W1; W2], bias b):
  h_ij = LeakyReLU(x_i @ W1 + (x_j - x_i) @ W2 + b)      j in kNN(i)
       = LeakyReLU(p_i + q_j),  p = x@(W1-W2)+b, q = x@W2
  out_i = max_j h_ij = LeakyReLU(p_i + max_j q_j)        (LeakyReLU monotone)

Per EdgeConv row-tile [128 x 4096]:
  (1) distance matmul on PE in fp32r (augmented lhsT=[2x^T;1], rhs=[x^T;-|x|^2],
      ranking-equivalent to the reference's neg_dist), with a -BIG*I matmul
      accumulated onto the diagonal block so self never ranks.
  (2) two-level exact top-19 on DVE: per-512-chunk max8+max_index (64
      candidates w/ indices), then narrow 64-wide rounds: 3x(max8 +
      match_replace) marks the top-19 candidate slots; compare-marked slots
      select their global indices, which are extracted by value with 3 more
      narrow max8 rounds (indices come out index-sorted; order is irrelevant
      for the max fold).
  (3) 19 single-offset-per-partition indirect DMA gathers of q rows
      (gpsimd swdge; ~1.09us each, payload-size independent), self's q row
      copied from SBUF, then a max fold tree.
  (4) h = LeakyReLU(p + m) via max(x, 0.01x); PE transpose into x^T for the
      next layer.

Sharding: data-parallel over batch, 1 sample per NeuronCore (8 cores).
Bedrock image: Q7 GPSIMD library instrs (dma_gather/ap_gather) unavailable;
multi-offset indirect DMA mis-lowers; InstIndirectCopy works but is ~27ns/elem
(too slow); DMA CCE compute ops don't compile. Only [P,1]-offset indirect DMA
works for gathers.
"""

import sys

sys.path.insert(0, "/opt/trn_rl_repo")

import numpy as np

import concourse.bass as bass
import concourse.bacc as bacc
import concourse.mybir as mybir
import concourse.tile as tile

N = 4096          # points per sample
P = 128           # partition tile (points per row-tile)
RT = N // P       # 32 row-tiles
KNN = 20          # neighbors (incl self)
NG = KNN - 1      # gathered neighbors (self handled from SBUF)
NCH = 8           # L1 chunks
CW = N // NCH     # 512 chunk width
NC2 = NCH * 8     # 64 candidates
CO = 64           # feature channels
F32 = mybir.dt.float32
F32R = mybir.dt.float32r
BF16 = mybir.dt.bfloat16
U16 = mybir.dt.uint16
I32 = mybir.dt.int32

AL = mybir.AluOpType
USE_F32R = False   # fp32r distance matmuls (4x faster PE, ~tf32 precision)
MMD = F32R if USE_F32R else F32
NEG = -1e30
BIG = 1e30


def _edge_conv(nc, tc, cst, pools, A, Bt, rqp, q_dram, xTn, proj=None):
    """One EdgeConv layer. A [Kd, N] = [2x^T; 1], Bt [Kd, N] = [x^T; -sq].
    rqp [Kd, 128] = [rq | rp]. Writes xTn [CO, N] = LeakyReLU(p+m)^T.
    If proj is given (final layer), also emits the projection per row-tile."""
    sb, sm, ps_s, ps_sm = pools

    # ---- head: q|p = x @ [W2 | W1-W2] ; q rows to DRAM + SBUF, p to SBUF
    q_sb = cst["q_sb"]
    p_sb = cst["p_sb"]
    for i in range(RT):
        isl = slice(i * P, (i + 1) * P)
        qp = ps_sm.tile([P, 2 * CO], F32, tag="sm")
        nc.tensor.matmul(qp[:], lhsT=A[:, isl], rhs=rqp[:], start=True, stop=True)
        nc.scalar.copy(q_sb[:, i, :], qp[:, 0:CO])
        nc.scalar.copy(p_sb[:, i, :], qp[:, CO:2 * CO])
        nc.sync.dma_start(q_dram[isl, :], q_sb[:, i, :])

    # ---- row-tile loop
    for i in range(RT):
        isl = slice(i * P, (i + 1) * P)

        # distances s_I [P, N] via PE fp32r in 4 PSUM quarters -> SBUF (scalar)
        # diagonal block gets -BIG*I accumulated so self never ranks.
        dq, doff = divmod(i * P, 1024)
        s_sb = sb.tile([P, N], F32, tag="s_sb")
        for qi in range(4):
            pq = ps_s.tile([P, 1024], F32, tag="ps_s")
            for c in range(2):
                col = qi * 1024 + c * 512
                nc.tensor.matmul(
                    pq[:, c * 512:(c + 1) * 512],
                    lhsT=A[:, isl],
                    rhs=Bt[:, col:col + 512],
                    start=True, stop=(qi != dq),
                )
            if qi == dq:
                nc.tensor.matmul(
                    pq[:, doff:doff + P],
                    lhsT=cst["negbig_ident"][:], rhs=cst["ident_bf"][:],
                    start=False, stop=True, skip_group_check=True,
                )
            nc.scalar.copy(s_sb[:, qi * 1024:(qi + 1) * 1024], pq[:])

        # L1: per-chunk top-8 values + local indices
        cv = sm.tile([P, NC2], F32, tag="cv")
        ci = sm.tile([P, NC2], U16, tag="ci")
        for c in range(NCH):
            c8 = slice(c * 8, (c + 1) * 8)
            cs = slice(c * CW, (c + 1) * CW)
            nc.vector.max(out=cv[:, c8], in_=s_sb[:, cs])
            nc.vector.max_index(out=ci[:, c8], in_max=cv[:, c8], in_values=s_sb[:, cs])

        # global candidate indices + 1 (f32)
        gidx1 = sm.tile([P, NC2], F32, tag="gidx1")
        cif = sm.tile([P, NC2], F32, tag="cif")
        nc.vector.tensor_copy(cif[:], ci[:])
        nc.vector.tensor_tensor(out=gidx1[:], in0=cif[:], in1=cst["offs1"][:],
                                op=AL.add)

        # L2: mark top-19 candidate slots via 3 match_replace rounds
        cvw = sm.tile([P, NC2], F32, tag="cvw")
        nc.vector.tensor_copy(cvw[:], cv[:])
        v8 = sm.tile([P, 3, 8], F32, tag="v8")
        nc.vector.max(out=v8[:, 0, :], in_=cvw[:])
        nc.vector.match_replace(out=cvw[:], in_to_replace=v8[:, 0, :],
                                in_values=cvw[:], imm_value=NEG)
        nc.vector.max(out=v8[:, 1, :], in_=cvw[:])
        nc.vector.match_replace(out=cvw[:], in_to_replace=v8[:, 1, :],
                                in_values=cvw[:], imm_value=NEG)
        nc.vector.max(out=v8[:, 2, :], in_=cvw[:])
        v3m = sm.tile([P, 8], F32, tag="v3m")
        nc.vector.memset(v3m[:], BIG)
        nc.vector.tensor_copy(v3m[:, 0:3], v8[:, 2, 0:3])
        nc.vector.match_replace(out=cvw[:], in_to_replace=v3m[:],
                                in_values=cvw[:], imm_value=NEG)

        # marked slots -> their (global index + 1); others -> 0
        sel = sm.tile([P, NC2], F32, tag="sel")
        nc.vector.tensor_tensor(out=sel[:], in0=cv[:], in1=cvw[:], op=AL.not_equal)
        midx = sm.tile([P, NC2], F32, tag="midx")
        nc.vector.tensor_tensor(out=midx[:], in0=sel[:], in1=gidx1[:], op=AL.mult)

        # extract the 19 marked indices by value (desc index order)
        ext = sm.tile([P, 3, 8], F32, tag="ext")
        nc.vector.max(out=ext[:, 0, :], in_=midx[:])
        nc.vector.match_replace(out=midx[:], in_to_replace=ext[:, 0, :],
                                in_values=midx[:], imm_value=0.0)
        nc.vector.max(out=ext[:, 1, :], in_=midx[:])
        nc.vector.match_replace(out=midx[:], in_to_replace=ext[:, 1, :],
                                in_values=midx[:], imm_value=0.0)
        nc.vector.max(out=ext[:, 2, :], in_=midx[:])
        jf = sm.tile([P, NG], F32, tag="jf")
        nc.vector.tensor_scalar(out=jf[:], in0=ext[:].rearrange("p a b -> p (a b)")[:, 0:NG],
                                scalar1=-1.0, scalar2=None, op0=AL.add)
        j32 = sm.tile([P, NG], I32, tag="j32")
        nc.vector.tensor_copy(j32[:], jf[:])

        # gather q rows of the 19 non-self neighbors; self q from SBUF
        gath = sm.tile([P, KNN, CO], F32, tag="gath")
        for k in range(NG):
            nc.gpsimd.indirect_dma_start(
                out=gath[:, k, :], out_offset=None, in_=q_dram[:],
                in_offset=bass.IndirectOffsetOnAxis(ap=j32[:, k:k + 1], axis=0),
            )
        nc.scalar.copy(gath[:, NG, :], q_sb[:, i, :])
        # max fold over 20
        for (a0, a1, n) in ((0, 10, 10), (0, 5, 5), (0, 2, 2), (0, 1, 1)):
            nc.vector.tensor_tensor(
                out=gath[:, a0:a0 + n, :], in0=gath[:, a0:a0 + n, :],
                in1=gath[:, a1:a1 + n, :], op=AL.max,
            )
        nc.vector.tensor_tensor(
            out=gath[:, 0:1, :], in0=gath[:, 0:1, :], in1=gath[:, 4:5, :], op=AL.max,
        )

        # h = LeakyReLU(p + m)
        hs = sm.tile([P, CO], F32, tag="hs")
        nc.vector.tensor_tensor(
            out=hs[:], in0=p_sb[:, i, :],
            in1=gath[:].rearrange("p a b -> p (a b)")[:, 0:CO], op=AL.add,
        )
        h = sm.tile([P, CO], F32, tag="h")
        nc.vector.scalar_tensor_tensor(
            out=h[:], in0=hs[:], scalar=0.01, in1=hs[:], op0=AL.mult, op1=AL.max,
        )

        # transpose into xTn[:, isl]
        tp = ps_sm.tile([CO, P], F32, tag="sm")
        nc.tensor.transpose(out=tp[:], in_=h[:], identity=cst["ident"][:])
        nc.scalar.copy(xTn[:, isl], tp[:])

        if proj is not None:
            out_dram, A1, x1T, x2T, wpx, wp1, wp2, wp3 = proj
            pj = ps_sm.tile([P, CO], F32, tag="sm")
            nc.tensor.matmul(pj[:], lhsT=A1[:, isl], rhs=wpx[:], start=True, stop=False)
            nc.tensor.matmul(pj[:], lhsT=x1T[:, isl], rhs=wp1[:], start=False, stop=False)
            nc.tensor.matmul(pj[:], lhsT=x2T[:, isl], rhs=wp2[:], start=False, stop=False)
            nc.tensor.matmul(pj[:], lhsT=xTn[:, isl], rhs=wp3[:], start=False, stop=True)
            po = sm.tile([P, CO], F32, tag="po")
            nc.scalar.copy(po[:], pj[:])
            nc.sync.dma_start(out_dram[isl, :], po[:])


def _build_a_bt(nc, tc, cst, pools, xT, A, Bt):
    """A = [2 xT; ones], Bt = [xT; -sum(xT^2, axis=C)] for a [CO, N] xT."""
    sb, sm, ps_s, ps_sm = pools
    nc.scalar.mul(A[0:CO, :], xT[:], 2.0)
    nc.vector.memset(A[CO:CO + 1, :].bitcast(F32), 1.0)
    nc.scalar.copy(Bt[0:CO, :], xT[:])
    for c in range(8):
        xsq = sm.tile([CO, 512], F32, tag="xsq")
        nc.scalar.square(xsq[:], xT[:, c * 512:(c + 1) * 512])
        sq = ps_sm.tile([1, 512], F32, tag="sm")
        nc.tensor.matmul(
            sq[:], lhsT=cst["ones64"][:], rhs=xsq[:],
            start=True, stop=True,
        )
        nc.scalar.mul(Bt[CO:CO + 1, c * 512:(c + 1) * 512], sq[:], -1.0)


def build_nc():
    nc = bacc.Bacc("TRN2", target_bir_lowering=False, debug=False)

    ins = {}
    for name, shape in [
        ("A1", [4, N]), ("Bt1", [4, N]),
        ("rqp1", [4, 2 * CO]), ("rqp2", [CO + 1, 2 * CO]), ("rqp3", [CO + 1, 2 * CO]),
        ("wpx", [4, CO]), ("wp1", [CO, CO]), ("wp2", [CO, CO]), ("wp3", [CO, CO]),
        ("ident", [P, P]),
    ]:
        ins[name] = nc.dram_tensor(name, shape, F32, kind="ExternalInput")
    out_dram = nc.dram_tensor("out", [N, CO], F32, kind="ExternalOutput")

    q_drams = [
        nc.dram_tensor(f"q_scr{i}", [N, CO], F32, kind="Internal") for i in range(3)
    ]

    from contextlib import ExitStack

    with tile.TileContext(nc) as tc, ExitStack() as ctx:
        cpool = ctx.enter_context(tc.tile_pool(name="const", bufs=1))
        sb = ctx.enter_context(tc.tile_pool(name="sb", bufs=2))
        sm = ctx.enter_context(tc.tile_pool(name="sm", bufs=2))
        ps_s = ctx.enter_context(tc.tile_pool(name="ps_s", bufs=2, space="PSUM"))
        ps_sm = ctx.enter_context(tc.tile_pool(name="ps_sm", bufs=4, space="PSUM"))
        pools = (sb, sm, ps_s, ps_sm)

        cst = {}
        cst["ones64"] = cpool.tile([CO, 1], F32, tag="ones64", name="ones64")
        nc.vector.memset(cst["ones64"][:], 1.0)

        # load inputs to SBUF
        st = {}
        for name in ins:
            t = cpool.tile(list(ins[name].shape), F32, tag=f"in_{name}",
                           name=f"in_{name}")
            nc.sync.dma_start(t[:], ins[name][:])
            st[name] = t
        cst["ident"] = st["ident"]

        # bf16 identity + -BIG identity for the diagonal kill
        ident_bf = cpool.tile([P, P], BF16, tag="ident_bf", name="ident_bf")
        nc.vector.tensor_copy(ident_bf[:], st["ident"][:])
        negbig_ident = cpool.tile([P, P], BF16, tag="negbig_ident",
                                  name="negbig_ident")
        nc.vector.tensor_scalar(out=negbig_ident[:], in0=st["ident"][:],
                                scalar1=NEG, scalar2=None, op0=AL.mult)
        cst["ident_bf"] = ident_bf
        cst["negbig_ident"] = negbig_ident

        # chunk offsets + 1 constant [P, 64]: col c*8+s = c*512 + 1
        offs1 = cpool.tile([P, NC2], F32, tag="offs1", name="offs1")
        for c in range(NCH):
            nc.vector.memset(offs1[:, c * 8:(c + 1) * 8], float(c * CW + 1))
        cst["offs1"] = offs1

        cst["q_sb"] = cpool.tile([P, RT, CO], F32, tag="q_sb", name="q_sb")
        cst["p_sb"] = cpool.tile([P, RT, CO], F32, tag="p_sb", name="p_sb")

        # fp32r copies of the head rhs (fp32r matmul inputs must be rounded)
        rqp_r = {}
        for lay, kd in ((1, 4), (2, CO + 1), (3, CO + 1)):
            t = cpool.tile([kd, 2 * CO], MMD, tag=f"rqp{lay}r", name=f"rqp{lay}r")
            nc.scalar.copy(t[:], st[f"rqp{lay}"][:])
            rqp_r[lay] = t

        x1T = cpool.tile([CO, N], F32, tag="x1T", name="x1T")
        x2T = cpool.tile([CO, N], F32, tag="x2T", name="x2T")
        x3T = cpool.tile([CO, N], F32, tag="x3T", name="x3T")
        A = cpool.tile([CO + 1, N], MMD, tag="A", name="A")
        Bt = cpool.tile([CO + 1, N], MMD, tag="Bt", name="Bt")

        # layer 1 augmented inputs into rows 0..3 of A/Bt (fp32r rounding copy)
        nc.scalar.copy(A[0:4, :], st["A1"][:])
        nc.scalar.copy(Bt[0:4, :], st["Bt1"][:])
        _edge_conv(nc, tc, cst, pools, A[0:4, :], Bt[0:4, :],
                   rqp_r[1][:], q_drams[0], x1T[:])

        _build_a_bt(nc, tc, cst, pools, x1T[:], A[:], Bt[:])
        _edge_conv(nc, tc, cst, pools, A[:], Bt[:],
                   rqp_r[2][:], q_drams[1], x2T[:])

        _build_a_bt(nc, tc, cst, pools, x2T[:], A[:], Bt[:])
        _edge_conv(nc, tc, cst, pools, A[:], Bt[:],
                   rqp_r[3][:], q_drams[2], x3T[:],
                   proj=(out_dram, st["A1"][:], x1T[:], x2T[:],
                         st["wpx"][:], st["wp1"][:], st["wp2"][:], st["wp3"][:]))

    nc.compile()
    return nc


def host_prep(x, w1, b1, w2, b2, w3, b3, wp, bp):
    """Per-sample A1/Bt1 plus shared weight-derived rhs tensors."""
    f = np.float32
    B = x.shape[0]
    shared = {}
    for lay, (w, b) in enumerate([(w1, b1), (w2, b2), (w3, b3)], start=1):
        C = w.shape[0] // 2
        W1, W2 = w[:C], w[C:]
        rq = np.concatenate([W2 * 0.5, np.zeros((1, CO), f)], axis=0)
        rp = np.concatenate([(W1 - W2) * 0.5, b[None, :]], axis=0)
        shared[f"rqp{lay}"] = np.ascontiguousarray(
            np.concatenate([rq, rp], axis=1), f)
    shared["wpx"] = np.concatenate(
        [wp[0:3] * 0.5, np.zeros((1, CO), f)], axis=0
    ).astype(f)
    shared["wp1"] = np.ascontiguousarray(wp[3:67], f)
    shared["wp2"] = np.ascontiguousarray(wp[67:131], f)
    shared["wp3"] = np.ascontiguousarray(wp[131:195], f)
    shared["ident"] = np.eye(P, dtype=f)

    per_core = []
    for b_i in range(B):
        xT = np.ascontiguousarray(x[b_i].T, f)            # [3, N]
        sq = np.sum(xT * xT, axis=0, keepdims=True)       # [1, N]
        A1 = np.concatenate([2.0 * xT, np.ones((1, N), f)], axis=0).astype(f)
        Bt1 = np.concatenate([xT, -sq], axis=0).astype(f)
        m = dict(shared)
        m["A1"] = A1
        m["Bt1"] = Bt1
        per_core.append(m)
    return per_core


_NC_CACHE = {}
TRACE = [False]
LAST_RESULT = {}


def kernel(**inputs):
    from concourse.bass_utils import run_bass_kernel_spmd

    x = np.asarray(inputs["x"], np.float32)
    args = [np.asarray(inputs[n], np.float32) for n in
            ("w1", "b1", "w2", "b2", "w3", "b3", "wp", "bp")]
    bp = args[-1]
    assert int(inputs["k"]) == KNN

    in_maps = host_prep(x, *args)

    if "nc" not in _NC_CACHE:
        _NC_CACHE["nc"] = build_nc()
    nc = _NC_CACHE["nc"]

    res = run_bass_kernel_spmd(
        nc, in_maps, core_ids=list(range(len(in_maps))), trace=TRACE[0]
    )
    LAST_RESULT["res"] = res
    out = np.stack([r["out"] for r in res.results], axis=0)  # [B, N, CO]
    return (out + bp[None, None, :]).astype(np.float32)
